# revision 7
# baseline (speedup 1.0000x reference)
"""Distributed attention block for Trainium2 (8 NeuronCores, SPMD).

Problem: B=2, S=2048, D=512, H=8 (head_dim = D = 512).
  qkv = einsum('bsd,dhf->bshf', x, w_qkv) + b_qkv     f = 3*D
  q, k, v = split(qkv); weights = softmax(q @ k^T / sqrt(D))
  out = einsum('bqhd,hdo->bqo', weights @ v, w_out) + b_out

Sharding: head-parallel (one head per core). Each core computes its head's
QKV projection, full attention for both batches, and its head's partial
output projection; a per-batch ReduceScatter sums the 8 partial outputs and
leaves each core with a 64-row feature shard that the host concatenates.

All on-chip layouts are feature-major ("transposed"), which makes every
matmul operand land in its natural layout with zero on-chip transposes:
  Q^T,K^T [d, t] <- stationary w-chunk, moving x^T
  V       [t, d] <- stationary x^T-block, moving w_v
  S^T     [k, q] <- stationary K^T-chunk, moving Q^T  (softmax over partitions)
  O^T     [d, q] <- stationary V-block, moving E^T
  Y^T     [o, t] <- stationary w_out-chunk, moving O^T
Softmax skips max-subtraction (scores have stddev ~0.2 for this problem's
scale-0.02 weights; exp is computed in f32 from PSUM). Row-sums come from an
all-ones stationary matmul accumulated in PSUM; the reciprocal is broadcast
across partitions for free because every PSUM row of that matmul holds the
same sums. Normalization is fused into the PV eviction multiply.
"""
import sys

for _p in ("/opt/trn_rl_repo",):
    if _p not in sys.path:
        sys.path.append(_p)

import numpy as np
import ml_dtypes

import concourse.bass as bass
import concourse.bacc as bacc
import concourse.mybir as mybir
import concourse.tile as tile
from concourse.bass import ts
from concourse.bass_utils import run_bass_kernel_spmd

BF16 = mybir.dt.bfloat16
F32 = mybir.dt.float32

B, S, D, H = 2, 2048, 512, 8
T = B * S                  # 4096 tokens
P = 128                    # partitions
NC = 8                     # cores
DC = D // P                # 4 contraction chunks of 128
FB = 512                   # moving free-dim per matmul
OUT_ROWS = D // NC         # 64 output-feature rows per core after RS
SCALE = float(D) ** -0.5

_CACHED = {}


def _no_cc():
    import os
    return bool(os.environ.get("KERNEL_NO_CC"))


def _build(s=S, debug=False):
    t_all = B * s
    nc = bacc.Bacc(None, target_bir_lowering=False, debug=debug, num_devices=NC)

    xt_ext = nc.declare_dram_parameter("xt", [D, t_all], BF16, isOutput=False)
    wq_ext = nc.declare_dram_parameter("wq", [D, D], BF16, isOutput=False)
    wk_ext = nc.declare_dram_parameter("wk", [D, D], BF16, isOutput=False)
    wv_ext = nc.declare_dram_parameter("wv", [D, D], BF16, isOutput=False)
    wo_ext = nc.declare_dram_parameter("wo", [D, D], BF16, isOutput=False)
    bq_ext = nc.declare_dram_parameter("bq", [P, DC], F32, isOutput=False)
    bk_ext = nc.declare_dram_parameter("bk", [P, DC], F32, isOutput=False)
    bv_ext = nc.declare_dram_parameter("bv", [D], F32, isOutput=False)
    bo_ext = nc.declare_dram_parameter("bo", [OUT_ROWS, 1], F32, isOutput=False)
    out_ext = nc.declare_dram_parameter("out", [OUT_ROWS, t_all], F32, isOutput=True)

    with tile.TileContext(nc) as tc:
        with (
            tc.tile_pool(name="consts", bufs=1) as consts,
            tc.tile_pool(name="qkv_sb", bufs=1) as qkv_sb,
            tc.tile_pool(name="et_sb", bufs=2) as et_pool,
            tc.tile_pool(name="small", bufs=2) as small,
            tc.tile_pool(name="ysb", bufs=3) as ysb_pool,
            tc.tile_pool(name="fin", bufs=2) as fin_pool,
            tc.tile_pool(name="ps_qkv", bufs=2, space="PSUM") as ps_qkv,
            tc.tile_pool(name="ps_st", bufs=2, space="PSUM") as ps_st,
            tc.tile_pool(name="ps_sum", bufs=1, space="PSUM") as ps_sum,
            tc.tile_pool(name="ps_o", bufs=2, space="PSUM") as ps_o,
            tc.tile_pool(name="ps_y", bufs=1, space="PSUM") as ps_y,
            tc.tile_pool(name="dram", bufs=1, space="DRAM") as dram,
        ):
            # ---- resident inputs -------------------------------------------------
            xt_sb = consts.tile([P, DC, t_all], BF16)           # x^T, 32KB/part
            for c in range(DC):
                nc.sync.dma_start(xt_sb[:, c, :], xt_ext[ts(c, P), :])
            wq_sb = consts.tile([P, DC, D], BF16)
            wk_sb = consts.tile([P, DC, D], BF16)
            wv_sb = consts.tile([P, DC, D], BF16)
            wo_sb = consts.tile([P, DC, D], BF16)
            for w_sb, w_ext in ((wq_sb, wq_ext), (wk_sb, wk_ext),
                                (wv_sb, wv_ext), (wo_sb, wo_ext)):
                for c in range(DC):
                    nc.sync.dma_start(w_sb[:, c, :], w_ext[ts(c, P), :])
            bq_sb = consts.tile([P, DC], F32)
            bk_sb = consts.tile([P, DC], F32)
            nc.sync.dma_start(bq_sb[:], bq_ext[:])
            nc.sync.dma_start(bk_sb[:], bk_ext[:])
            # V bias broadcast along partitions (same bias for every token row)
            bv_sb = consts.tile([P, D], F32)
            nc.sync.dma_start(
                bv_sb[:],
                bass.AP(tensor=bv_ext, offset=0, ap=[[0, P], [1, D]]),
            )
            bo_sb = consts.tile([OUT_ROWS, 1], F32)
            nc.sync.dma_start(bo_sb[:], bo_ext[:])
            ones_sb = consts.tile([P, P], BF16)
            nc.vector.memset(ones_sb[:], 1.0)

            # ---- per-batch working tiles (shared slots across batches) ----------
            # [P, DC, s]: feature-major Q^T / K^T for one batch
            qt_sb = qkv_sb.tile([P, DC, s], BF16, tag="qt")
            kt_sb = qkv_sb.tile([P, DC, s], BF16, tag="kt")
            # [P, kb, D]: token-major V for one batch (16 key-blocks of 128)
            v_sb = qkv_sb.tile([P, s // P, D], BF16, tag="v")
            # [P, DC, s]: feature-major attention output O^T for one batch
            ot_sb = qkv_sb.tile([P, DC, s], BF16, tag="ot")

            y_bounce = [dram.tile([D, s], F32, name=f"y_bounce{b}") for b in range(B)]
            rs_out = [dram.tile([OUT_ROWS, s], F32, name=f"rs_out{b}") for b in range(B)]

            def qkv_phase(b):
                t0 = b * s
                # Q^T / K^T: psum [f=128, t=512] = w_chunk.T @ x^T
                for w_sb, bias_sb, dst in ((wq_sb, bq_sb, qt_sb), (wk_sb, bk_sb, kt_sb)):
                    for f in range(DC):
                        for t in range(s // FB):
                            ps = ps_qkv.tile([P, FB], F32, tag="ps_qkv")
                            for c in range(DC):
                                nc.tensor.matmul(
                                    ps[:], w_sb[:, c, ts(f, P)],
                                    xt_sb[:, c, t0 + t * FB: t0 + (t + 1) * FB],
                                    start=(c == 0), stop=(c == DC - 1),
                                )
                            nc.scalar.activation(
                                dst[:, f, ts(t, FB)], ps[:],
                                mybir.ActivationFunctionType.Identity,
                                bias=bias_sb[:, f:f + 1],
                            )
                # V: psum [t=128, dv=512] = x^T-block.T @ w_v
                for kb in range(s // P):
                    ps = ps_qkv.tile([P, D], F32, tag="ps_qkv")
                    for c in range(DC):
                        nc.tensor.matmul(
                            ps[:], xt_sb[:, c, t0 + kb * P: t0 + (kb + 1) * P],
                            wv_sb[:, c, :],
                            start=(c == 0), stop=(c == DC - 1),
                        )
                    nc.vector.tensor_add(v_sb[:, kb, :], ps[:], bv_sb[:])

            def attn_phase(b):
                for qb in range(s // FB):
                    et_sb = et_pool.tile([P, s // P, FB], BF16, tag="et")
                    ps_s = ps_sum.tile([P, FB], F32, tag="ps_sum")
                    for kb in range(s // P):
                        ps = ps_st.tile([P, FB], F32, tag="ps_st")
                        for c in range(DC):
                            nc.tensor.matmul(
                                ps[:], kt_sb[:, c, ts(kb, P)],
                                qt_sb[:, c, ts(qb, FB)],
                                start=(c == 0), stop=(c == DC - 1),
                            )
                        # exp(scale * s) straight out of PSUM (f32) into bf16
                        nc.scalar.activation(
                            et_sb[:, kb, :], ps[:],
                            mybir.ActivationFunctionType.Exp, scale=SCALE,
                        )
                        # rowsum accumulation: every PSUM row gets the same sums
                        nc.tensor.matmul(
                            ps_s[:], ones_sb[:], et_sb[:, kb, :],
                            start=(kb == 0), stop=(kb == s // P - 1),
                        )
                    brecip = small.tile([P, FB], F32, tag="brecip")
                    nc.vector.reciprocal(brecip[:], ps_s[:])
                    # PV: psum [dv=128, q=512] = V-block.T @ E^T, normalize on evict
                    for dv in range(DC):
                        ps = ps_o.tile([P, FB], F32, tag="ps_o")
                        for kb in range(s // P):
                            nc.tensor.matmul(
                                ps[:], v_sb[:, kb, ts(dv, P)], et_sb[:, kb, :],
                                start=(kb == 0), stop=(kb == s // P - 1),
                            )
                        nc.vector.tensor_mul(
                            ot_sb[:, dv, ts(qb, FB)], ps[:], brecip[:],
                        )

            def outproj_phase(b):
                for ob in range(DC):
                    for t in range(s // FB):
                        ps = ps_y.tile([P, FB], F32, tag="ps_y")
                        for c in range(DC):
                            nc.tensor.matmul(
                                ps[:], wo_sb[:, c, ts(ob, P)],
                                ot_sb[:, c, ts(t, FB)],
                                start=(c == 0), stop=(c == DC - 1),
                            )
                        y_sb = ysb_pool.tile([P, FB], F32, tag="y_sb")
                        nc.vector.tensor_copy(y_sb[:], ps[:])
                        nc.sync.dma_start(
                            y_bounce[b][ts(ob, P), ts(t, FB)], y_sb[:])
                if _no_cc():
                    nc.sync.dma_start(
                        rs_out[b][:], y_bounce[b][0:OUT_ROWS, :])
                else:
                    nc.gpsimd.collective_compute(
                        "ReduceScatter",
                        mybir.AluOpType.add,
                        replica_groups=[list(range(NC))],
                        ins=[y_bounce[b].opt()],
                        outs=[rs_out[b].opt()],
                    )

            def finalize(b):
                fin = fin_pool.tile([OUT_ROWS, s], F32, tag="fin")
                nc.sync.dma_start(fin[:], rs_out[b][:])
                nc.scalar.activation(
                    fin[:], fin[:],
                    mybir.ActivationFunctionType.Identity, bias=bo_sb[:],
                )
                nc.sync.dma_start(out_ext[:, b * s:(b + 1) * s], fin[:])

            qkv_phase(0)
            attn_phase(0)
            outproj_phase(0)      # RS(b0) overlaps attn(b1)
            qkv_phase(1)
            attn_phase(1)
            outproj_phase(1)
            finalize(0)
            finalize(1)

    nc.compile()
    return nc


def _get_nc():
    if "nc" not in _CACHED:
        _CACHED["nc"] = _build()
    return _CACHED["nc"]


def _marshal(x, w_qkv, b_qkv, w_out, b_out):
    x = np.asarray(x)
    w_qkv = np.asarray(w_qkv)
    b_qkv = np.asarray(b_qkv)
    w_out = np.asarray(w_out)
    b_out = np.asarray(b_out)

    bf = ml_dtypes.bfloat16
    xt = np.ascontiguousarray(x.reshape(T, D).T).astype(bf)
    in_maps = []
    for h in range(NC):
        wq = np.ascontiguousarray(w_qkv[:, h, 0:D]).astype(bf)
        wk = np.ascontiguousarray(w_qkv[:, h, D:2 * D]).astype(bf)
        wv = np.ascontiguousarray(w_qkv[:, h, 2 * D:3 * D]).astype(bf)
        wo = np.ascontiguousarray(w_out[h]).astype(bf)
        bq = np.ascontiguousarray(
            b_qkv[h, 0:D].astype(np.float32).reshape(DC, P).T)
        bk = np.ascontiguousarray(
            b_qkv[h, D:2 * D].astype(np.float32).reshape(DC, P).T)
        bv = np.ascontiguousarray(b_qkv[h, 2 * D:3 * D].astype(np.float32))
        bo = np.ascontiguousarray(
            b_out[h * OUT_ROWS:(h + 1) * OUT_ROWS]
            .astype(np.float32).reshape(OUT_ROWS, 1))
        in_maps.append({
            "xt": xt, "wq": wq, "wk": wk, "wv": wv, "wo": wo,
            "bq": bq, "bk": bk, "bv": bv, "bo": bo,
        })
    return in_maps


def kernel(x, w_qkv, b_qkv, w_out, b_out):
    x = np.asarray(x)
    in_maps = _marshal(x, w_qkv, b_qkv, w_out, b_out)
    nc = _get_nc()
    res = run_bass_kernel_spmd(nc, in_maps, core_ids=list(range(NC)))
    yt = np.concatenate([res.results[i]["out"] for i in range(NC)], axis=0)
    return np.ascontiguousarray(yt.T).reshape(B, S, D).astype(x.dtype)


# revision 10
# speedup vs baseline: 1.0274x; 1.0274x over previous
"""Distributed attention block for Trainium2 (8 NeuronCores, SPMD).

Problem: B=2, S=2048, D=512, H=8 (head_dim = D = 512).
  qkv = einsum('bsd,dhf->bshf', x, w_qkv) + b_qkv     f = 3*D
  q, k, v = split(qkv); weights = softmax(q @ k^T / sqrt(D))
  out = einsum('bqhd,hdo->bqo', weights @ v, w_out) + b_out

Sharding: head-parallel (one head per core). Each core computes its head's
QKV projection, full attention for both batches, and its head's partial
output projection; chunked ReduceScatters sum the 8 partial outputs and
leave each core with a 64-row feature shard that the host concatenates
(output bias is applied host-side during unsharding).

All on-chip layouts are feature-major ("transposed"), which makes every
matmul operand land in its natural layout with zero on-chip transposes:
  Q^T,K^T [d, t] <- stationary w-chunk, moving x^T
  V       [t, d] <- stationary x^T-block, moving w_v
  S^T     [k, q] <- stationary K^T-chunk, moving Q^T  (softmax over partitions)
  O^T     [d, q] <- stationary V-block, moving E^T
  Y^T     [o, t] <- stationary w_out-chunk, moving O^T
Softmax skips max-subtraction (scores have stddev ~0.2 for this problem's
scale-0.02 weights; exp is computed in f32 from PSUM). Row-sums: a DVE add
tree over the 16 E^T tiles plus one all-ones f32 matmul for the final
cross-partition reduction (every PSUM row then holds the same sums, giving
the partition-broadcast reciprocal for free). Normalization is fused into
the PV eviction multiply.
"""
import sys

for _p in ("/opt/trn_rl_repo",):
    if _p not in sys.path:
        sys.path.append(_p)

import numpy as np
import ml_dtypes

import concourse.bass as bass
import concourse.bacc as bacc
import concourse.mybir as mybir
import concourse.tile as tile
from concourse.bass import ts
from concourse.bass_utils import run_bass_kernel_spmd

BF16 = mybir.dt.bfloat16
F32 = mybir.dt.float32

B, S, D, H = 2, 2048, 512, 8
T = B * S                  # 4096 tokens
P = 128                    # partitions
NC = 8                     # cores
DC = D // P                # 4 contraction chunks of 128
FB = 512                   # moving free-dim per matmul
OUT_ROWS = D // NC         # 64 output-feature rows per core after RS
SCALE = float(D) ** -0.5

_CACHED = {}


def _build(s=S, debug=False):
    t_all = B * s
    tch = s // FB              # outproj token chunks per batch
    nc = bacc.Bacc(None, target_bir_lowering=False, debug=debug, num_devices=NC)

    xt_ext = nc.declare_dram_parameter("xt", [D, t_all], BF16, isOutput=False)
    wq_ext = nc.declare_dram_parameter("wq", [D, D], BF16, isOutput=False)
    wk_ext = nc.declare_dram_parameter("wk", [D, D], BF16, isOutput=False)
    wv_ext = nc.declare_dram_parameter("wv", [D, D], BF16, isOutput=False)
    wo_ext = nc.declare_dram_parameter("wo", [D, D], BF16, isOutput=False)
    bq_ext = nc.declare_dram_parameter("bq", [P, DC], F32, isOutput=False)
    bk_ext = nc.declare_dram_parameter("bk", [P, DC], F32, isOutput=False)
    bv_ext = nc.declare_dram_parameter("bv", [D], F32, isOutput=False)
    out_ext = nc.declare_dram_parameter("out", [OUT_ROWS, t_all], F32, isOutput=True)

    with tile.TileContext(nc) as tc:
        with (
            tc.tile_pool(name="consts", bufs=1) as consts,
            tc.tile_pool(name="qkv_sb", bufs=1) as qkv_sb,
            tc.tile_pool(name="et_sb", bufs=2) as et_pool,
            tc.tile_pool(name="small", bufs=2) as small,
            tc.tile_pool(name="tsum_sb", bufs=1) as tsum_pool,
            tc.tile_pool(name="ysb", bufs=3) as ysb_pool,
            tc.tile_pool(name="ps_qkv", bufs=2, space="PSUM") as ps_qkv,
            tc.tile_pool(name="ps_st", bufs=2, space="PSUM") as ps_st,
            tc.tile_pool(name="ps_sum", bufs=1, space="PSUM") as ps_sum,
            tc.tile_pool(name="ps_o", bufs=2, space="PSUM") as ps_o,
            tc.tile_pool(name="ps_y", bufs=1, space="PSUM") as ps_y,
            tc.tile_pool(name="dram", bufs=1, space="DRAM") as dram,
        ):
            # ---- resident inputs ------------------------------------------------
            # x^T loaded token-chunk-major so the first QKV matmuls start early
            xt_sb = consts.tile([P, DC, t_all], BF16)
            for t in range(t_all // FB):
                for c in range(DC):
                    nc.sync.dma_start(xt_sb[:, c, ts(t, FB)],
                                      xt_ext[ts(c, P), ts(t, FB)])
            wq_sb = consts.tile([P, DC, D], BF16)
            wk_sb = consts.tile([P, DC, D], BF16)
            wv_sb = consts.tile([P, DC, D], BF16)
            wo_sb = consts.tile([P, DC, D], BF16)
            for w_sb, w_ext in ((wq_sb, wq_ext), (wk_sb, wk_ext),
                                (wv_sb, wv_ext), (wo_sb, wo_ext)):
                for c in range(DC):
                    nc.sync.dma_start(w_sb[:, c, :], w_ext[ts(c, P), :])
            bq_sb = consts.tile([P, DC], F32)
            bk_sb = consts.tile([P, DC], F32)
            nc.sync.dma_start(bq_sb[:], bq_ext[:])
            nc.sync.dma_start(bk_sb[:], bk_ext[:])
            # V bias broadcast along partitions (same bias for every token row)
            bv_sb = consts.tile([P, D], F32)
            nc.sync.dma_start(
                bv_sb[:],
                bass.AP(tensor=bv_ext, offset=0, ap=[[0, P], [1, D]]),
            )
            ones_sb = consts.tile([P, P], BF16)
            nc.vector.memset(ones_sb[:], 1.0)

            # ---- per-batch working tiles (shared slots across batches) ---------
            qt_sb = qkv_sb.tile([P, DC, s], BF16, tag="qt")
            kt_sb = qkv_sb.tile([P, DC, s], BF16, tag="kt")
            v_sb = qkv_sb.tile([P, s // P, D], BF16, tag="v")
            ot_sb = qkv_sb.tile([P, DC, s], BF16, tag="ot")

            y_ch = [[dram.tile([D, FB], F32, name=f"y_ch{b}_{t}")
                     for t in range(tch)] for b in range(B)]
            rs_ch = [[dram.tile([OUT_ROWS, FB], F32, name=f"rs_ch{b}_{t}")
                      for t in range(tch)] for b in range(B)]

            def qkv_phase(b):
                t0 = b * s
                # Q^T / K^T: psum [f=128, t=512] = w_chunk.T @ x^T
                for w_sb, bias_sb, dst in ((wq_sb, bq_sb, qt_sb), (wk_sb, bk_sb, kt_sb)):
                    for f in range(DC):
                        for t in range(s // FB):
                            ps = ps_qkv.tile([P, FB], F32, tag="ps_qkv")
                            for c in range(DC):
                                nc.tensor.matmul(
                                    ps[:], w_sb[:, c, ts(f, P)],
                                    xt_sb[:, c, t0 + t * FB: t0 + (t + 1) * FB],
                                    start=(c == 0), stop=(c == DC - 1),
                                )
                            nc.vector.tensor_scalar_add(
                                dst[:, f, ts(t, FB)], ps[:], bias_sb[:, f:f + 1])
                # V: psum [t=128, dv=512] = x^T-block.T @ w_v
                for kb in range(s // P):
                    ps = ps_qkv.tile([P, D], F32, tag="ps_qkv")
                    for c in range(DC):
                        nc.tensor.matmul(
                            ps[:], xt_sb[:, c, t0 + kb * P: t0 + (kb + 1) * P],
                            wv_sb[:, c, :],
                            start=(c == 0), stop=(c == DC - 1),
                        )
                    nc.vector.tensor_add(v_sb[:, kb, :], ps[:], bv_sb[:])

            def attn_phase(b):
                nkb = s // P
                for qb in range(s // FB):
                    et_sb = et_pool.tile([P, nkb, FB], BF16, tag="et")
                    for kb in range(nkb):
                        ps = ps_st.tile([P, FB], F32, tag="ps_st")
                        for c in range(DC):
                            nc.tensor.matmul(
                                ps[:], kt_sb[:, c, ts(kb, P)],
                                qt_sb[:, c, ts(qb, FB)],
                                start=(c == 0), stop=(c == DC - 1),
                            )
                        # exp(scale * s) straight out of PSUM (f32) into bf16
                        nc.scalar.activation(
                            et_sb[:, kb, :], ps[:],
                            mybir.ActivationFunctionType.Exp, scale=SCALE,
                        )
                    # rowsum: DVE add-tree over the kb axis (f32 partials,
                    # bf16 final level) + one bf16 all-ones matmul for the
                    # cross-partition reduction
                    tsum = tsum_pool.tile([P, nkb // 2, FB], F32, tag="tsum")
                    tsum_bf = tsum_pool.tile([P, FB], BF16, tag="tsum_bf")
                    half = nkb // 2
                    nc.vector.tensor_add(
                        tsum[:, 0:half, :], et_sb[:, 0:half, :],
                        et_sb[:, half:nkb, :])
                    while half > 2:
                        h2 = half // 2
                        nc.vector.tensor_add(
                            tsum[:, 0:h2, :], tsum[:, 0:h2, :],
                            tsum[:, h2:half, :])
                        half = h2
                    nc.vector.tensor_add(
                        tsum_bf[:], tsum[:, 0, :], tsum[:, 1, :])
                    ps_s = ps_sum.tile([P, FB], F32, tag="ps_sum")
                    nc.tensor.matmul(ps_s[:], ones_sb[:], tsum_bf[:],
                                     start=True, stop=True)
                    brecip = small.tile([P, FB], F32, tag="brecip")
                    nc.vector.reciprocal(brecip[:], ps_s[:])
                    # PV: psum [dv=128, q=512] = V-block.T @ E^T, normalize on evict
                    for dv in range(DC):
                        ps = ps_o.tile([P, FB], F32, tag="ps_o")
                        for kb in range(nkb):
                            nc.tensor.matmul(
                                ps[:], v_sb[:, kb, ts(dv, P)], et_sb[:, kb, :],
                                start=(kb == 0), stop=(kb == nkb - 1),
                            )
                        nc.vector.tensor_mul(
                            ot_sb[:, dv, ts(qb, FB)], ps[:], brecip[:],
                        )

            def outproj_phase(b):
                for t in range(tch):
                    for ob in range(DC):
                        ps = ps_y.tile([P, FB], F32, tag="ps_y")
                        for c in range(DC):
                            nc.tensor.matmul(
                                ps[:], wo_sb[:, c, ts(ob, P)],
                                ot_sb[:, c, ts(t, FB)],
                                start=(c == 0), stop=(c == DC - 1),
                            )
                        y_sb = ysb_pool.tile([P, FB], F32, tag="y_sb")
                        nc.vector.tensor_copy(y_sb[:], ps[:])
                        nc.sync.dma_start(y_ch[b][t][ts(ob, P), :], y_sb[:])
                    nc.gpsimd.collective_compute(
                        "ReduceScatter",
                        mybir.AluOpType.add,
                        replica_groups=[list(range(NC))],
                        ins=[y_ch[b][t].opt()],
                        outs=[rs_ch[b][t].opt()],
                    )

            with nc.named_scope("qkv0"):
                qkv_phase(0)
            with nc.named_scope("attn0"):
                attn_phase(0)
            with nc.named_scope("out0"):
                outproj_phase(0)      # RS(b0) chunks overlap batch-1 compute
            with nc.named_scope("qkv1"):
                qkv_phase(1)
            with nc.named_scope("attn1"):
                attn_phase(1)
            with nc.named_scope("out1"):
                outproj_phase(1)
            # final DRAM->DRAM copies of the reduce-scattered shards; emitted
            # last so their collective-completion waits can't block anything
            with nc.named_scope("fin"):
                for b in range(B):
                    for t in range(tch):
                        nc.sync.dma_start(
                            out_ext[:, b * s + t * FB: b * s + (t + 1) * FB],
                            rs_ch[b][t][:])

    nc.compile()
    return nc


def _get_nc():
    if "nc" not in _CACHED:
        _CACHED["nc"] = _build()
    return _CACHED["nc"]


def _marshal(x, w_qkv, b_qkv, w_out, b_out):
    x = np.asarray(x)
    w_qkv = np.asarray(w_qkv)
    b_qkv = np.asarray(b_qkv)
    w_out = np.asarray(w_out)

    bf = ml_dtypes.bfloat16
    xt = np.ascontiguousarray(x.reshape(T, D).T).astype(bf)
    in_maps = []
    for h in range(NC):
        wq = np.ascontiguousarray(w_qkv[:, h, 0:D]).astype(bf)
        wk = np.ascontiguousarray(w_qkv[:, h, D:2 * D]).astype(bf)
        wv = np.ascontiguousarray(w_qkv[:, h, 2 * D:3 * D]).astype(bf)
        wo = np.ascontiguousarray(w_out[h]).astype(bf)
        bq = np.ascontiguousarray(
            b_qkv[h, 0:D].astype(np.float32).reshape(DC, P).T)
        bk = np.ascontiguousarray(
            b_qkv[h, D:2 * D].astype(np.float32).reshape(DC, P).T)
        bv = np.ascontiguousarray(b_qkv[h, 2 * D:3 * D].astype(np.float32))
        in_maps.append({
            "xt": xt, "wq": wq, "wk": wk, "wv": wv, "wo": wo,
            "bq": bq, "bk": bk, "bv": bv,
        })
    return in_maps


def kernel(x, w_qkv, b_qkv, w_out, b_out):
    x = np.asarray(x)
    b_out = np.asarray(b_out)
    in_maps = _marshal(x, w_qkv, b_qkv, w_out, b_out)
    nc = _get_nc()
    res = run_bass_kernel_spmd(nc, in_maps, core_ids=list(range(NC)))
    yt = np.concatenate([res.results[i]["out"] for i in range(NC)], axis=0)
    yt = yt + b_out.astype(np.float32).reshape(D, 1)
    return np.ascontiguousarray(yt.T).reshape(B, S, D).astype(x.dtype)


# revision 11
# speedup vs baseline: 1.2087x; 1.1765x over previous
"""Distributed attention block for Trainium2 (8 NeuronCores, SPMD).

Problem: B=2, S=2048, D=512, H=8 (head_dim = D = 512).
  qkv = einsum('bsd,dhf->bshf', x, w_qkv) + b_qkv     f = 3*D
  q, k, v = split(qkv); weights = softmax(q @ k^T / sqrt(D))
  out = einsum('bqhd,hdo->bqo', weights @ v, w_out) + b_out

Sharding: head-parallel (one head per core). Each core computes its head's
QKV projection, full attention for both batches, and its head's partial
output projection; chunked ReduceScatters sum the 8 partial outputs and
leave each core with a 64-row feature shard that the host concatenates
(output bias is applied host-side during unsharding).

All on-chip layouts are feature-major ("transposed"), which makes every
matmul operand land in its natural layout with zero on-chip transposes:
  Q^T,K^T [d, t] <- stationary w-chunk, moving x^T
  V       [t, d] <- stationary x^T-block, moving w_v
  S^T     [k, q] <- stationary K^T-chunk, moving Q^T  (softmax over partitions)
  O^T     [d, q] <- stationary V-block, moving E^T
  Y^T     [o, t] <- stationary w_out-chunk, moving O^T
Softmax skips max-subtraction (scores have stddev ~0.2 for this problem's
scale-0.02 weights; exp is computed in f32 from PSUM). Row-sums: a DVE add
tree over the 16 E^T tiles plus one all-ones f32 matmul for the final
cross-partition reduction (every PSUM row then holds the same sums, giving
the partition-broadcast reciprocal for free). Normalization is fused into
the PV eviction multiply.
"""
import sys

for _p in ("/opt/trn_rl_repo",):
    if _p not in sys.path:
        sys.path.append(_p)

import numpy as np
import ml_dtypes

import concourse.bass as bass
import concourse.bacc as bacc
import concourse.mybir as mybir
import concourse.tile as tile
from concourse.bass import ts
from concourse.bass_utils import run_bass_kernel_spmd

BF16 = mybir.dt.bfloat16
F32 = mybir.dt.float32

B, S, D, H = 2, 2048, 512, 8
T = B * S                  # 4096 tokens
P = 128                    # partitions
NC = 8                     # cores
DC = D // P                # 4 contraction chunks of 128
FB = 512                   # moving free-dim per matmul
OUT_ROWS = D // NC         # 64 output-feature rows per core after RS
SCALE = float(D) ** -0.5

_CACHED = {}


def _build(s=S, debug=False):
    t_all = B * s
    tch = s // FB              # outproj token chunks per batch
    nc = bacc.Bacc(None, target_bir_lowering=False, debug=debug, num_devices=NC)

    xt_ext = nc.declare_dram_parameter("xt", [D, t_all], BF16, isOutput=False)
    wq_ext = nc.declare_dram_parameter("wq", [D, D], BF16, isOutput=False)
    wk_ext = nc.declare_dram_parameter("wk", [D, D], BF16, isOutput=False)
    wv_ext = nc.declare_dram_parameter("wv", [D, D], BF16, isOutput=False)
    wo_ext = nc.declare_dram_parameter("wo", [D, D], BF16, isOutput=False)
    bq_ext = nc.declare_dram_parameter("bq", [P, DC], F32, isOutput=False)
    bk_ext = nc.declare_dram_parameter("bk", [P, DC], F32, isOutput=False)
    bv_ext = nc.declare_dram_parameter("bv", [D], F32, isOutput=False)
    out_ext = nc.declare_dram_parameter("out", [OUT_ROWS, t_all], F32, isOutput=True)

    with tile.TileContext(nc) as tc:
        with (
            tc.tile_pool(name="consts", bufs=1) as consts,
            tc.tile_pool(name="qkv_sb", bufs=1) as qkv_sb,
            tc.tile_pool(name="et_sb", bufs=2) as et_pool,
            tc.tile_pool(name="small", bufs=2) as small,
            tc.tile_pool(name="tsum_sb", bufs=1) as tsum_pool,
            tc.tile_pool(name="ysb", bufs=3) as ysb_pool,
            tc.tile_pool(name="ps_qkv", bufs=2, space="PSUM") as ps_qkv,
            tc.tile_pool(name="ps_st", bufs=2, space="PSUM") as ps_st,
            tc.tile_pool(name="ps_sum", bufs=1, space="PSUM") as ps_sum,
            tc.tile_pool(name="ps_o", bufs=2, space="PSUM") as ps_o,
            tc.tile_pool(name="ps_y", bufs=1, space="PSUM") as ps_y,
            tc.tile_pool(name="dram", bufs=1, space="DRAM") as dram,
        ):
            # ---- resident inputs ------------------------------------------------
            xt_sb = consts.tile([P, DC, t_all], BF16)
            wq_sb = consts.tile([P, DC, D], BF16)
            wk_sb = consts.tile([P, DC, D], BF16)
            wv_sb = consts.tile([P, DC, D], BF16)
            wo_sb = consts.tile([P, DC, D], BF16)
            for w_sb, w_ext in ((wq_sb, wq_ext), (wk_sb, wk_ext),
                                (wv_sb, wv_ext), (wo_sb, wo_ext)):
                for c in range(DC):
                    nc.sync.dma_start(w_sb[:, c, :], w_ext[ts(c, P), :])
            bq_sb = consts.tile([P, DC], F32)
            bk_sb = consts.tile([P, DC], F32)
            nc.sync.dma_start(bq_sb[:], bq_ext[:])
            nc.sync.dma_start(bk_sb[:], bk_ext[:])
            # V bias broadcast along partitions (same bias for every token row)
            bv_sb = consts.tile([P, D], F32)
            nc.sync.dma_start(
                bv_sb[:],
                bass.AP(tensor=bv_ext, offset=0, ap=[[0, P], [1, D]]),
            )
            ones_sb = consts.tile([P, P], BF16)
            nc.vector.memset(ones_sb[:], 1.0)

            # x^T loaded token-chunk-major, after the (smaller) weight loads
            # so the first QKV matmuls start as early as possible
            for t in range(t_all // FB):
                for c in range(DC):
                    nc.sync.dma_start(xt_sb[:, c, ts(t, FB)],
                                      xt_ext[ts(c, P), ts(t, FB)])

            # ---- per-batch working tiles (shared slots across batches) ---------
            qt_sb = qkv_sb.tile([P, DC, s], BF16, tag="qt")
            kt_sb = qkv_sb.tile([P, DC, s], BF16, tag="kt")
            v_sb = qkv_sb.tile([P, s // P, D], BF16, tag="v")
            ot_sb = qkv_sb.tile([P, DC, s], BF16, tag="ot")

            y_ch = [[dram.tile([D, FB], F32, name=f"y_ch{b}_{t}")
                     for t in range(tch)] for b in range(B)]
            rs_ch = [[dram.tile([OUT_ROWS, FB], F32, name=f"rs_ch{b}_{t}")
                      for t in range(tch)] for b in range(B)]

            def qkv_phase(b):
                t0 = b * s
                # Q^T / K^T: psum [f=128, t=512] = w_chunk.T @ x^T
                for w_sb, bias_sb, dst in ((wq_sb, bq_sb, qt_sb), (wk_sb, bk_sb, kt_sb)):
                    for f in range(DC):
                        for t in range(s // FB):
                            ps = ps_qkv.tile([P, FB], F32, tag="ps_qkv")
                            for c in range(DC):
                                nc.tensor.matmul(
                                    ps[:], w_sb[:, c, ts(f, P)],
                                    xt_sb[:, c, t0 + t * FB: t0 + (t + 1) * FB],
                                    start=(c == 0), stop=(c == DC - 1),
                                )
                            nc.vector.tensor_scalar_add(
                                dst[:, f, ts(t, FB)], ps[:], bias_sb[:, f:f + 1])
                # V: psum [t=128, dv=512] = x^T-block.T @ w_v
                for kb in range(s // P):
                    ps = ps_qkv.tile([P, D], F32, tag="ps_qkv")
                    for c in range(DC):
                        nc.tensor.matmul(
                            ps[:], xt_sb[:, c, t0 + kb * P: t0 + (kb + 1) * P],
                            wv_sb[:, c, :],
                            start=(c == 0), stop=(c == DC - 1),
                        )
                    nc.vector.tensor_add(v_sb[:, kb, :], ps[:], bv_sb[:])

            def attn_phase(b, fuse_outproj=True):
                nkb = s // P
                for qb in range(s // FB):
                    et_sb = et_pool.tile([P, nkb, FB], BF16, tag="et")
                    # pairwise partial rowsums, emitted as the exps complete
                    epair = tsum_pool.tile([P, nkb // 2, FB], BF16, tag="epair")
                    for kb in range(nkb):
                        ps = ps_st.tile([P, FB], F32, tag="ps_st")
                        for c in range(DC):
                            nc.tensor.matmul(
                                ps[:], kt_sb[:, c, ts(kb, P)],
                                qt_sb[:, c, ts(qb, FB)],
                                start=(c == 0), stop=(c == DC - 1),
                            )
                        # exp(scale * s) straight out of PSUM (f32) into bf16
                        nc.scalar.activation(
                            et_sb[:, kb, :], ps[:],
                            mybir.ActivationFunctionType.Exp, scale=SCALE,
                        )
                        if kb % 2 == 1:
                            nc.vector.tensor_add(
                                epair[:, kb // 2, :], et_sb[:, kb - 1, :],
                                et_sb[:, kb, :])
                    # cross-partition rowsum via accumulated all-ones matmuls
                    ps_s = ps_sum.tile([P, FB], F32, tag="ps_sum")
                    for j in range(nkb // 2):
                        nc.tensor.matmul(ps_s[:], ones_sb[:], epair[:, j, :],
                                         start=(j == 0), stop=(j == nkb // 2 - 1))
                    brecip = small.tile([P, FB], F32, tag="brecip")
                    nc.vector.reciprocal(brecip[:], ps_s[:])
                    # PV: psum [dv=128, q=512] = V-block.T @ E^T, normalize on evict
                    for dv in range(DC):
                        ps = ps_o.tile([P, FB], F32, tag="ps_o")
                        for kb in range(nkb):
                            nc.tensor.matmul(
                                ps[:], v_sb[:, kb, ts(dv, P)], et_sb[:, kb, :],
                                start=(kb == 0), stop=(kb == nkb - 1),
                            )
                        nc.vector.tensor_mul(
                            ot_sb[:, dv, ts(qb, FB)], ps[:], brecip[:],
                        )
                    if fuse_outproj:
                        outproj_chunk(b, qb)

            def outproj_chunk(b, t):
                for ob in range(DC):
                    ps = ps_y.tile([P, FB], F32, tag="ps_y")
                    for c in range(DC):
                        nc.tensor.matmul(
                            ps[:], wo_sb[:, c, ts(ob, P)],
                            ot_sb[:, c, ts(t, FB)],
                            start=(c == 0), stop=(c == DC - 1),
                        )
                    y_sb = ysb_pool.tile([P, FB], F32, tag="y_sb")
                    nc.vector.tensor_copy(y_sb[:], ps[:])
                    nc.sync.dma_start(y_ch[b][t][ts(ob, P), :], y_sb[:])
                nc.gpsimd.collective_compute(
                    "ReduceScatter",
                    mybir.AluOpType.add,
                    replica_groups=[list(range(NC))],
                    ins=[y_ch[b][t].opt()],
                    outs=[rs_ch[b][t].opt()],
                )

            with nc.named_scope("qkv0"):
                qkv_phase(0)
            with nc.named_scope("attn0"):
                attn_phase(0)         # outproj+RS chunks fused per q-block
            with nc.named_scope("qkv1"):
                qkv_phase(1)
            with nc.named_scope("attn1"):
                attn_phase(1)
            # final DRAM->DRAM copies of the reduce-scattered shards; emitted
            # last so their collective-completion waits can't block anything
            with nc.named_scope("fin"):
                for b in range(B):
                    for t in range(tch):
                        nc.sync.dma_start(
                            out_ext[:, b * s + t * FB: b * s + (t + 1) * FB],
                            rs_ch[b][t][:])

    nc.compile()
    return nc


def _get_nc():
    if "nc" not in _CACHED:
        _CACHED["nc"] = _build()
    return _CACHED["nc"]


def _marshal(x, w_qkv, b_qkv, w_out, b_out):
    x = np.asarray(x)
    w_qkv = np.asarray(w_qkv)
    b_qkv = np.asarray(b_qkv)
    w_out = np.asarray(w_out)

    bf = ml_dtypes.bfloat16
    xt = np.ascontiguousarray(x.reshape(T, D).T).astype(bf)
    in_maps = []
    for h in range(NC):
        wq = np.ascontiguousarray(w_qkv[:, h, 0:D]).astype(bf)
        wk = np.ascontiguousarray(w_qkv[:, h, D:2 * D]).astype(bf)
        wv = np.ascontiguousarray(w_qkv[:, h, 2 * D:3 * D]).astype(bf)
        wo = np.ascontiguousarray(w_out[h]).astype(bf)
        bq = np.ascontiguousarray(
            b_qkv[h, 0:D].astype(np.float32).reshape(DC, P).T)
        bk = np.ascontiguousarray(
            b_qkv[h, D:2 * D].astype(np.float32).reshape(DC, P).T)
        bv = np.ascontiguousarray(b_qkv[h, 2 * D:3 * D].astype(np.float32))
        in_maps.append({
            "xt": xt, "wq": wq, "wk": wk, "wv": wv, "wo": wo,
            "bq": bq, "bk": bk, "bv": bv,
        })
    return in_maps


def kernel(x, w_qkv, b_qkv, w_out, b_out):
    x = np.asarray(x)
    b_out = np.asarray(b_out)
    in_maps = _marshal(x, w_qkv, b_qkv, w_out, b_out)
    nc = _get_nc()
    res = run_bass_kernel_spmd(nc, in_maps, core_ids=list(range(NC)))
    yt = np.concatenate([res.results[i]["out"] for i in range(NC)], axis=0)
    yt = yt + b_out.astype(np.float32).reshape(D, 1)
    return np.ascontiguousarray(yt.T).reshape(B, S, D).astype(x.dtype)
